# revision 6
# baseline (speedup 1.0000x reference)
"""Trainium2 Bass kernel for the ConformanceGNN (3-layer bipartite GNN message passing).

Sharding: dest-sharded edge parallelism over 8 cores. Each core owns a slice of the
destination nodes per direction, gathers source-node rows (feat|score|bad packed in
512B fp16 table rows) with indirect DMA, segment-sums via one-hot matmuls into PSUM,
applies the fused update (message MLP folded into derived weights host-side), and
AllGathers the updated node table for the next pass. Global softmax normalizer Z and
max-shift m are computed densely on every core from the replicated score column.
"""
import numpy as np

P, T, H, L = 50000, 20000, 128, 3
NC = 8
PSL, PSLP = P // NC, 6400          # place slice, padded (50 tiles)
TSL, TSLP = T // NC, 2560          # trans slice, padded (20 tiles)
PTILES, TTILES = PSLP // 128, TSLP // 128
PROWS, TROWS = NC * PSLP, NC * TSLP  # replicated table rows


def _patch_tile_drain():
    """walrus here rejects >1 sem wait on the closing Drain; split waits onto NOPs."""
    import concourse.tile as tile
    import concourse.mybir as mybir
    from concourse.vector_clock import ScopedClock

    def _drain_and_barrier_split(self, tick_clock, wait_clock):
        probe = self.nc.sync.nop(nofuse=True)
        wait_clock.add_sem_waits(probe.ins, ScopedClock({None: tick_clock.global_clock}))
        si = probe.ins.sync_info
        waits = list(si.on_wait) if si is not None else []
        if len(waits) > 1:
            probe.ins.sync_info = mybir.SyncInfo(on_wait=waits[:1], on_update=[])
            for w in waits[1:]:
                n = self.nc.sync.nop(nofuse=True)
                n.ins.sync_info = mybir.SyncInfo(on_wait=[w], on_update=[])
        self.nc.sync.drain()
        self.nc.all_engine_barrier()
        assert self.sems is not None
        popped = self.nc._tile_sem_poison_stack.pop()
        assert popped is self._sem_poison
        self.nc.clear_and_free_semaphores(list(self.sems.allocated().values()))
        self.nc.all_engine_barrier()

    tile.TileContext._drain_and_barrier = _drain_and_barrier_split

    if getattr(tile.TileContext, "_wait_split_patched", False):
        return
    orig_commit = tile.TileContext._commit_instruction

    def _commit_split(self, inst, lazy_reg_writes=True):
        si = getattr(inst, "sync_info", None)
        if (si is not None and si.on_wait and len(si.on_wait) > 1
                and inst.engine != mybir.EngineType.Unassigned):
            waits = list(si.on_wait)
            inst.sync_info = mybir.SyncInfo(on_wait=[waits[-1]], on_update=list(si.on_update))
            cb = self.nc._state.pop_inst_callback()
            try:
                eng = self.nc.engines[inst.engine]
                for w in waits[:-1]:
                    n = eng.nop(nofuse=True)
                    n.ins.sync_info = mybir.SyncInfo(on_wait=[w], on_update=[])
            finally:
                self.nc._state.push_inst_callback(cb)
        orig_commit(self, inst, lazy_reg_writes)

    tile.TileContext._commit_instruction = _commit_split
    tile.TileContext._wait_split_patched = True


def _prep_edges(src, dst, src_slp, dst_sl, dst_slp, src_rowmap):
    """Per-core chunked edge lists. Returns idx [NC][128, NCHtot] i32 (table rows),
    dloc [NC][128, NCHtot] f32 (dest pos in tile, -1 pad), nch [ntiles]."""
    ntiles = dst_slp // 128
    per_core = []
    for c in range(NC):
        m = (dst >= c * dst_sl) & (dst < (c + 1) * dst_sl)
        es, ed = src[m], dst[m] - c * dst_sl
        tiles = [[] for _ in range(ntiles)]
        order = np.argsort(ed // 128, kind="stable")
        for e in order:
            tiles[ed[e] // 128].append((es[e], ed[e] % 128))
        per_core.append(tiles)
    nch = np.array([max(1, max((len(per_core[c][t]) + 127) // 128 for c in range(NC)))
                    for t in range(ntiles)])
    tot = int(nch.sum())
    idx = np.zeros((NC, 128, tot), np.int32)
    dloc = np.full((NC, 128, tot), -1.0, np.float32)
    for c in range(NC):
        k0 = 0
        for t in range(ntiles):
            lst = per_core[c][t]
            for j, (s, dp) in enumerate(lst):
                idx[c, j % 128, k0 + j // 128] = src_rowmap(s)
                dloc[c, j % 128, k0 + j // 128] = dp
            k0 += nch[t]
    return idx, dloc, nch


def kernel(**inputs):
    import concourse.bass as bass
    import concourse.mybir as mybir
    import concourse.tile as tile
    from concourse.bass_utils import run_bass_kernel_spmd
    _patch_tile_drain()
    f16, f32, i32 = mybir.dt.float16, mybir.dt.float32, mybir.dt.int32
    AF = mybir.ActivationFunctionType
    ALU = mybir.AluOpType

    d = {k: np.asarray(v) for k, v in inputs.items()}

    # ---- host: derived weights -------------------------------------------------
    def f16a(x):
        return np.ascontiguousarray(x, np.float16)

    def f32a(x):
        return np.ascontiguousarray(x, np.float32)

    Wsq, bvecs, bcols, ucols = [], [], [], []
    for l in range(L):
        Wp2t, bp2t = d["Wp2t"][l], d["bp2t"][l]
        Wt2p, bt2p = d["Wt2p"][l], d["bt2p"][l]
        Wpu, bpu = d["Wpu"][l], d["bpu"][l]
        Wtu, btu = d["Wtu"][l], d["btu"][l]
        I = np.eye(H, dtype=np.float32)
        Wsq += [I + Wtu[:H], Wp2t @ Wtu[H:], I + Wpu[:H], Wt2p @ Wpu[H:]]
        bvecs += [bp2t @ Wtu[H:], bt2p @ Wpu[H:]]
        bcols += [btu, bpu]
        ucols.append(d["Wt2p"][l] @ d["Wpa"][l])          # u_post[l]: trans scores
    for l in (1, 2):
        ucols.append(d["Wp2t"][l] @ d["Wta"][l])          # u_pre[l]: place scores
    u_pre0 = d["Wp2t"][0] @ d["Wta"][0]
    alpha_p = float(d["W_pe"][0] @ u_pre0)
    beta_p = float(d["b_pe"] @ u_pre0)
    vt = d["W_te"] @ ucols[0]                              # [8] trans embed score
    beta_t = float(d["b_te"] @ ucols[0])

    # ---- host: edges / tables --------------------------------------------------
    pre, post = np.asarray(d["pre_edge_index"]), np.asarray(d["post_edge_index"])
    prow = lambda p: (p // PSL) * PSLP + p % PSL
    trow = lambda t: (t // TSL) * TSLP + t % TSL
    idxA, dlocA, nchA = _prep_edges(pre[0].astype(np.int64), pre[1].astype(np.int64),
                                    PSLP, TSL, TSLP, prow)
    idxB, dlocB, nchB = _prep_edges(post[0].astype(np.int64), post[1].astype(np.int64),
                                    TSLP, PSL, PSLP, trow)
    NA, NB = idxA.shape[2], idxB.shape[2]

    cntP = np.zeros(PROWS, np.float32)
    np.add.at(cntP, prow(pre[0].astype(np.int64)), 1.0)
    cntT = np.zeros(TROWS, np.float32)
    np.add.at(cntT, trow(post[0].astype(np.int64)), 1.0)
    CP, CT = PROWS // 128, TROWS // 128

    pf = d["place_features"].reshape(-1).astype(np.float32)
    tf = d["transition_features"].astype(np.float32)
    bad = (pf <= 0).astype(np.float16)

    per_core_in = []
    for c in range(NC):
        pfs = np.zeros(PSLP, np.float32)
        pfs[:PSL] = pf[c * PSL:(c + 1) * PSL]
        bads = np.zeros(PSLP, np.float16)
        bads[:PSL] = bad[c * PSL:(c + 1) * PSL]
        tfs = np.zeros((TSLP, 8), np.float32)
        tfs[:TSL] = tf[c * TSL:(c + 1) * TSL]
        W3sl = np.zeros((H, TSLP), np.float16)
        W3sl[:, :TSL] = d["W3"][:, c * TSL:(c + 1) * TSL].astype(np.float16)
        b3sl = np.zeros((1, TSLP), np.float16)
        b3sl[0, :TSL] = d["b3"][c * TSL:(c + 1) * TSL].astype(np.float16)
        Wc1nt = np.zeros((128, 20, 256), np.float16)
        Wc1en = np.zeros((128, 20, 256), np.float16)
        for p in range(128):
            for k in range(20):
                n = p * 20 + k
                if n < TSL:
                    Wc1nt[p, k] = d["Wc1"][3 * H + c * TSL + n].astype(np.float16)
                    Wc1en[p, k] = d["Wc1"][3 * H + T + c * TSL + n].astype(np.float16)
        m = {
            "idxA": idxA[c], "dlocA": f32a(dlocA[c]),
            "idxB": idxB[c], "dlocB": f32a(dlocB[c]),
            "cntP": f32a(cntP.reshape(128, CP)), "cntT": f32a(cntT.reshape(128, CT)),
            "pfrow": f16a(pfs.reshape(1, -1)), "pfcol": f32a(pfs.reshape(128, PTILES, order="F").reshape(128, PTILES)),
            "badc": f16a(bads.reshape(128, PTILES, order="F")),
            "tfT": f16a(tfs.T), "prefc": f16a(d["prefix_encoding"].reshape(-1, 1)),
            "iota": f16a(np.tile(np.arange(128, dtype=np.float16)[None, :], (128, 1))),
            "ident": f16a(np.eye(128)), "onesr": f16a(np.ones((1, 128))),
            "onesrf": f32a(np.ones((1, 128))), "one11": f16a(np.ones((1, 1))),
            "zcol": f16a(np.zeros((128, 1))),
            "pmaskp": f32a((np.arange(PSLP).reshape(PTILES, 128).T < PSL).astype(np.float32)),
            "pmaskt": f32a((np.arange(TSLP).reshape(TTILES, 128).T < TSL).astype(np.float32)),
            "Wsq": f16a(np.concatenate(Wsq, 1)), "bvecs": f16a(np.concatenate([b.reshape(1, -1) for b in bvecs], 1)),
            "bcols": f32a(np.stack(bcols, 1)), "ucols": f16a(np.stack(ucols, 1)),
            "Wpe": f16a(d["W_pe"]), "bpe": f16a(d["b_pe"].reshape(1, -1)),
            "Wte": f16a(d["W_te"]), "bte": f16a(d["b_te"].reshape(1, -1)),
            "vtc": f16a(vt.reshape(-1, 1)),
            "Wpp": f16a(d["Wpp"]), "bpp": f16a(d["bpp"].reshape(1, -1)),
            "Wtp": f16a(d["Wtp"]), "btp": f16a(d["btp"].reshape(1, -1)),
            "Wpx": f16a(d["W_px"]), "bpx": f16a(d["b_px"].reshape(1, -1)),
            "W1p": f16a(d["W1"].reshape(3, 128, 256).transpose(1, 0, 2).reshape(128, 768)),
            "b1r": f16a(d["b1"].reshape(1, -1)),
            "W2p": f16a(d["W2"].reshape(2, 128, 128).transpose(1, 0, 2).reshape(128, 256)),
            "b2r": f16a(d["b2"].reshape(1, -1)),
            "W3sl": W3sl, "b3sl": b3sl,
            "Wc1c": f16a(d["Wc1"][:384].reshape(3, 128, 256).transpose(1, 0, 2).reshape(128, 768)),
            "bc1r": f16a(d["bc1"].reshape(1, -1)),
            "Wc1nt": Wc1nt.reshape(128, 5120), "Wc1en": Wc1en.reshape(128, 5120),
            "Wc2p": f16a(d["Wc2"].reshape(2, 128, 128).transpose(1, 0, 2).reshape(128, 256)),
            "bc2r": f16a(d["bc2"].reshape(1, -1)),
            "Wc3c": f16a(d["Wc3"]), "bc3": f16a(d["bc3"].reshape(1, 1)),
        }
        per_core_in.append(m)

    # pfcol/badc pack: node n of slice at (p=n%128, t=n//128)
    for m in per_core_in:
        pfs = m["pfrow"].reshape(-1).astype(np.float32)
        m["pfcol"] = f32a(pfs.reshape(PTILES, 128).T)
        bads = m["badc"]  # placeholder fix below
    for c in range(NC):
        bads = np.zeros(PSLP, np.float16)
        bads[:PSL] = bad[c * PSL:(c + 1) * PSL]
        per_core_in[c]["badc"] = f16a(bads.reshape(PTILES, 128).T)

    # ---- bass program ----------------------------------------------------------
    nc = bass.Bass(num_devices=NC)
    ins = {}
    for k, v in per_core_in[0].items():
        dt = {np.dtype(np.float16): f16, np.dtype(np.float32): f32,
              np.dtype(np.int32): i32}[v.dtype]
        ins[k] = nc.dram_tensor(k, list(v.shape), dt, kind="ExternalInput")
    nt_o = nc.dram_tensor("nt", [1, TSLP], f32, kind="ExternalOutput")
    en_o = nc.dram_tensor("en", [1, TSLP], f32, kind="ExternalOutput")
    cf_o = nc.dram_tensor("cf", [1, 1], f32, kind="ExternalOutput")

    slabP = [nc.dram_tensor(f"slabP{i}", [PSLP, 256], f16) for i in range(4)]
    slabT = [nc.dram_tensor(f"slabT{i}", [TSLP, 256], f16) for i in range(4)]
    tabP = [nc.dram_tensor(f"tabP{i}", [PROWS, 256], f16) for i in range(3)]
    tabT = [nc.dram_tensor(f"tabT{i}", [TROWS, 256], f16) for i in range(3)]
    arm_i = nc.dram_tensor("arm_i", [128, 2], f32)
    arm_o = nc.dram_tensor("arm_o", [128, 2], f32)
    arc_i = nc.dram_tensor("arc_i", [1, 256], f32)
    arc_o = nc.dram_tensor("arc_o", [1, 256], f32)
    RG = [list(range(NC))]

    with tile.TileContext(nc) as tc:
        with (
            tc.tile_pool(name="pers", bufs=1) as pers,
            tc.tile_pool(name="sb", bufs=4) as sb,
            tc.tile_pool(name="sw", bufs=4) as sw,
            tc.tile_pool(name="ps", bufs=2, space="PSUM") as ps,
            tc.tile_pool(name="pacc", bufs=2, space="PSUM") as pacc,
        ):
            # persistent/consts
            def load(name, dt=None):
                t = pers.tile(list(per_core_in[0][name].shape), dt or ins[name].dtype, name=name)
                nc.sync.dma_start(out=t[:], in_=ins[name][:])
                return t
            iota, ident = load("iota"), load("ident")
            onesr, onesrf, one11, zcol = load("onesr"), load("onesrf"), load("one11"), load("zcol")
            idxA_s, dlocA_s = load("idxA"), load("dlocA")
            idxB_s, dlocB_s = load("idxB"), load("dlocB")
            pfrow, pfcol, badc, tfT = load("pfrow"), load("pfcol"), load("badc"), load("tfT")
            pmaskp, pmaskt = load("pmaskp"), load("pmaskt")
            Wsq_s, bvecs_s, bcols_s, ucols_s = load("Wsq"), load("bvecs"), load("bcols"), load("ucols")
            cntP_s, cntT_s = load("cntP"), load("cntT")
            viol = pers.tile([1, TSLP], f32)
            acc_p = pers.tile([128, 1], f32)
            acc_t = pers.tile([128, 1], f32)
            nc.vector.memset(acc_p[:], 0.0)
            nc.vector.memset(acc_t[:], 0.0)

            def row2col(row_ap, n=128):
                """[1,n] -> psum [n,1] via matmul with one11."""
                pt = ps.tile([128, 1], f32, space="PSUM", tag="mm")
                nc.tensor.matmul(pt[:n, :], lhsT=row_ap, rhs=one11[:], start=True, stop=True)
                return pt

            def bcast_col(v11_f32):
                """[1,1] f32 -> [128,1] f32 col (psum)."""
                pt = ps.tile([128, 1], f32, space="PSUM", tag="mm")
                nc.tensor.matmul(pt[:], lhsT=onesrf[:], rhs=v11_f32, start=True, stop=True)
                return pt

            # ---------- embeds ----------
            Wpe_s, bpe_s = load("Wpe"), load("bpe")
            for t in range(PTILES):
                pe = ps.tile([128, 128], f32, space="PSUM", tag="mm")
                nc.tensor.matmul(pe[:], lhsT=pfrow[0:1, t * 128:(t + 1) * 128], rhs=Wpe_s[:], start=True, stop=False)
                nc.tensor.matmul(pe[:], lhsT=onesr[:], rhs=bpe_s[:], start=False, stop=True)
                wt = sw.tile([128, 256], f16)
                nc.vector.tensor_copy(wt[:, 0:128], pe[:])
                nc.scalar.activation(wt[:, 128:129], pfcol[:, t:t + 1], AF.Copy, bias=beta_p, scale=alpha_p)
                nc.vector.tensor_copy(wt[:, 129:130], badc[:, t:t + 1])
                if (t + 1) * 128 > PSL:
                    nc.vector.tensor_scalar_mul(wt[:, 0:130], wt[:, 0:130], pmaskp[:, t:t + 1])
                nc.sync.dma_start(out=slabP[0][t * 128:(t + 1) * 128, :], in_=wt[:])
            Wte_s, bte_s, vtc_s = load("Wte"), load("bte"), load("vtc")
            for t in range(TTILES):
                pe = ps.tile([128, 128], f32, space="PSUM", tag="mm")
                nc.tensor.matmul(pe[:], lhsT=tfT[:, t * 128:(t + 1) * 128], rhs=Wte_s[:], start=True, stop=False)
                nc.tensor.matmul(pe[:], lhsT=onesr[:], rhs=bte_s[:], start=False, stop=True)
                sp = ps.tile([128, 1], f32, space="PSUM", tag="mm")
                nc.tensor.matmul(sp[:], lhsT=tfT[:, t * 128:(t + 1) * 128], rhs=vtc_s[:], start=True, stop=True)
                wt = sw.tile([128, 256], f16)
                nc.vector.tensor_copy(wt[:, 0:128], pe[:])
                nc.scalar.activation(wt[:, 128:129], sp[:], AF.Copy, bias=beta_t)
                nc.vector.tensor_copy(wt[:, 129:130], zcol[:])
                if (t + 1) * 128 > TSL:
                    nc.vector.tensor_scalar_mul(wt[:, 0:130], wt[:, 0:130], pmaskt[:, t:t + 1])
                nc.sync.dma_start(out=slabT[0][t * 128:(t + 1) * 128, :], in_=wt[:])
            nc.gpsimd.collective_compute("AllGather", ALU.bypass, replica_groups=RG,
                                         ins=[slabP[0][:]], outs=[tabP[0][:]])

            # ---------- passes ----------
            def mz(tab, cnt_s, C):
                """returns (mneg_col f32 sbuf [128,1], zinv_col f32 sbuf [128,1], zinv11 sbuf [1,1])"""
                s_sb = sb.tile([128, C], f16)
                nc.sync.dma_start(out=s_sb[:], in_=tab[:, 128:129])
                m11 = sb.tile([1, 1], f32)
                nc.gpsimd.tensor_reduce(m11[:], s_sb[:], mybir.AxisListType.XYZWC, ALU.max)
                mneg11 = sb.tile([1, 1], f32)
                nc.vector.tensor_scalar_mul(mneg11[:], m11[:], -1.0)
                mnc = sb.tile([128, 1], f32)
                nc.vector.tensor_copy(mnc[:], bcast_col(mneg11[:])[:])
                zt = sb.tile([128, C], f32)
                nc.scalar.activation(zt[:], s_sb[:], AF.Exp, bias=mnc[:])
                nc.vector.tensor_tensor(out=zt[:], in0=zt[:], in1=cnt_s[:], op=ALU.mult)
                z11 = sb.tile([1, 1], f32)
                nc.gpsimd.tensor_reduce(z11[:], zt[:], mybir.AxisListType.XYZWC, ALU.add)
                zi11 = sb.tile([1, 1], f32)
                nc.vector.reciprocal(zi11[:], z11[:])
                zic = sb.tile([128, 1], f32)
                nc.vector.tensor_copy(zic[:], bcast_col(zi11[:])[:])
                return mnc, zic, zi11

            def gnn_pass(l, is_pre):
                tab = tabP[l] if is_pre else tabT[l]
                cnt_s, C = (cntP_s, CP) if is_pre else (cntT_s, CT)
                idx_s, dloc_s, nch = (idxA_s, dlocA_s, nchA) if is_pre else (idxB_s, dlocB_s, nchB)
                ntl, SL = (TTILES, TSL) if is_pre else (PTILES, PSL)
                rslab, wslab = (slabT[l], slabT[l + 1]) if is_pre else (slabP[l], slabP[l + 1])
                wi = (4 * l + (0 if is_pre else 2))
                W1s = sb.tile([128, 128], f16)
                nc.sync.dma_start(out=W1s[:], in_=ins["Wsq"][:, wi * 128:(wi + 1) * 128])
                W2s = sb.tile([128, 128], f16)
                nc.sync.dma_start(out=W2s[:], in_=ins["Wsq"][:, (wi + 1) * 128:(wi + 2) * 128])
                bi = 2 * l + (0 if is_pre else 1)
                mnc, zic, zi11 = mz(tab, cnt_s, C)
                ui = l if is_pre else (2 + l)   # ucols index; post l=2 has none
                k0 = 0
                for t in range(ntl):
                    stf = pacc.tile([128, 128], f32, space="PSUM", tag="stf")
                    ext = pacc.tile([1, 128], f32, space="PSUM", tag="ext")
                    need_viol = is_pre and l == 0
                    if need_viol:
                        vlp = pacc.tile([1, 128], f32, space="PSUM", tag="viol", bufs=1)
                    for k in range(k0, k0 + int(nch[t])):
                        g = sb.tile([128, 256], f16)
                        nc.gpsimd.indirect_dma_start(
                            out=g[:], out_offset=None, in_=tab[:],
                            in_offset=bass.IndirectOffsetOnAxis(ap=idx_s[:, k:k + 1], axis=0))
                        w = sb.tile([128, 1], f32)
                        nc.scalar.activation(w[:], g[:, 128:129], AF.Exp, bias=mnc[:])
                        nc.vector.tensor_scalar_mul(g[:, 0:128], g[:, 0:128], w[:])
                        nc.vector.tensor_copy(g[:, 128:129], w[:])
                        meq = sb.tile([128, 128], f16)
                        nc.vector.tensor_scalar(out=meq[:], in0=iota[:], scalar1=dloc_s[:, k:k + 1],
                                                scalar2=None, op0=ALU.is_equal)
                        st, sp_ = (k == k0), (k == k0 + int(nch[t]) - 1)
                        nc.tensor.matmul(stf[:], lhsT=g[:, 0:128], rhs=meq[:], start=st, stop=sp_)
                        nc.tensor.matmul(ext[:], lhsT=g[:, 128:129], rhs=meq[:], start=st, stop=sp_)
                        if need_viol:
                            nc.tensor.matmul(vlp[:], lhsT=g[:, 129:130], rhs=meq[:], start=st, stop=sp_)
                    k0 += int(nch[t])
                    if need_viol:
                        nc.vector.tensor_copy(viol[0:1, t * 128:(t + 1) * 128], vlp[:])
                    # update
                    ld = sb.tile([128, 128], f16)
                    nc.sync.dma_start(out=ld[:], in_=rslab[t * 128:(t + 1) * 128, 0:128])
                    tp = ps.tile([128, 128], f32, space="PSUM", tag="mm")
                    nc.tensor.matmul(tp[:], lhsT=ld[:], rhs=ident[:], start=True, stop=True)
                    hT = sb.tile([128, 128], f16)
                    nc.vector.tensor_copy(hT[:], tp[:])
                    upd = ps.tile([128, 128], f32, space="PSUM", tag="mm")
                    nc.tensor.matmul(upd[:], lhsT=W1s[:], rhs=hT[:], start=True, stop=False)
                    sts = sb.tile([128, 128], f16)
                    nc.vector.tensor_scalar_mul(sts[:], stf[:], zic[:])
                    nc.tensor.matmul(upd[:], lhsT=W2s[:], rhs=sts[:], start=False, stop=False)
                    satt = sb.tile([1, 128], f16)
                    nc.vector.tensor_scalar_mul(satt[:], ext[:], zi11[:])
                    nc.tensor.matmul(upd[:], lhsT=bvecs_s[0:1, bi * 128:(bi + 1) * 128], rhs=satt[:],
                                     start=False, stop=True)
                    hN = sb.tile([128, 128], f16)
                    nc.scalar.activation(hN[:], upd[:], AF.Relu, bias=bcols_s[:, bi:bi + 1])
                    if (t + 1) * 128 > SL:
                        nc.vector.memset(hN[:, max(0, SL - t * 128):128], 0.0)
                    if l == 2:
                        rs = sb.tile([128, 1], f32)
                        nc.vector.tensor_reduce(rs[:], hN[:], mybir.AxisListType.X, ALU.add)
                        acc = acc_t if is_pre else acc_p
                        nc.vector.tensor_tensor(out=acc[:], in0=acc[:], in1=rs[:], op=ALU.add)
                    wp = ps.tile([128, 128], f32, space="PSUM", tag="mm")
                    nc.tensor.matmul(wp[:], lhsT=hN[:], rhs=ident[:], start=True, stop=True)
                    wt = sw.tile([128, 256], f16)
                    nc.vector.tensor_copy(wt[:, 0:128], wp[:])
                    if not (not is_pre and l == 2):
                        sp = ps.tile([128, 1], f32, space="PSUM", tag="mm")
                        nc.tensor.matmul(sp[:], lhsT=hN[:], rhs=ucols_s[:, ui:ui + 1], start=True, stop=True)
                        nc.vector.tensor_copy(wt[:, 128:129], sp[:])
                    else:
                        nc.vector.tensor_copy(wt[:, 128:129], zcol[:])
                    if is_pre:
                        nc.vector.tensor_copy(wt[:, 129:130], zcol[:])
                    else:
                        nc.vector.tensor_copy(wt[:, 129:130], badc[:, t:t + 1])
                    nc.sync.dma_start(out=wslab[t * 128:(t + 1) * 128, :], in_=wt[:])
                # allgather updated table
                if is_pre:
                    nc.gpsimd.collective_compute("AllGather", ALU.bypass, replica_groups=RG,
                                                 ins=[wslab[:]], outs=[tabT[l][:]])
                elif l < 2:
                    nc.gpsimd.collective_compute("AllGather", ALU.bypass, replica_groups=RG,
                                                 ins=[wslab[:]], outs=[tabP[l + 1][:]])

            for l in range(L):
                gnn_pass(l, True)
                gnn_pass(l, False)

            # ---------- head ----------
            mv = sb.tile([128, 2], f32)
            nc.vector.tensor_copy(mv[:, 0:1], acc_p[:])
            nc.vector.tensor_copy(mv[:, 1:2], acc_t[:])
            nc.sync.dma_start(out=arm_i[:], in_=mv[:])
            nc.gpsimd.collective_compute("AllReduce", ALU.add, replica_groups=RG,
                                         ins=[arm_i[:]], outs=[arm_o[:]])
            mvr = sb.tile([128, 2], f32)
            nc.sync.dma_start(out=mvr[:], in_=arm_o[:])
            mcols = sb.tile([128, 2], f16)
            nc.vector.tensor_scalar_mul(mcols[:, 0:1], mvr[:, 0:1], 1.0 / P)
            nc.vector.tensor_scalar_mul(mcols[:, 1:2], mvr[:, 1:2], 1.0 / T)
            Wpp_s, bpp_s, Wtp_s, btp_s = load("Wpp"), load("bpp"), load("Wtp"), load("btp")
            Wpx_s, bpx_s, prefc = load("Wpx"), load("bpx"), load("prefc")
            comb = sb.tile([1, 384], f16)
            pg = ps.tile([1, 128], f32, space="PSUM", tag="mm")
            nc.tensor.matmul(pg[:], lhsT=mcols[:, 0:1], rhs=Wpp_s[:], start=True, stop=False)
            nc.tensor.matmul(pg[:], lhsT=one11[:], rhs=bpp_s[:], start=False, stop=True)
            nc.vector.tensor_copy(comb[0:1, 0:128], pg[:])
            tg = ps.tile([1, 128], f32, space="PSUM", tag="mm")
            nc.tensor.matmul(tg[:], lhsT=mcols[:, 1:2], rhs=Wtp_s[:], start=True, stop=False)
            nc.tensor.matmul(tg[:], lhsT=one11[:], rhs=btp_s[:], start=False, stop=True)
            nc.vector.tensor_copy(comb[0:1, 128:256], tg[:])
            ph = ps.tile([1, 128], f32, space="PSUM", tag="mm")
            nc.tensor.matmul(ph[:], lhsT=prefc[:], rhs=Wpx_s[:], start=True, stop=False)
            nc.tensor.matmul(ph[:], lhsT=one11[:], rhs=bpx_s[:], start=False, stop=True)
            nc.vector.tensor_copy(comb[0:1, 256:384], ph[:])
            ccols = sb.tile([128, 3], f16)
            for k in range(3):
                nc.vector.tensor_copy(ccols[:, k:k + 1], row2col(comb[0:1, k * 128:(k + 1) * 128])[:])
            W1p_s, b1r_s = load("W1p"), load("b1r")
            h1p = ps.tile([1, 256], f32, space="PSUM", tag="mm")
            for k in range(3):
                nc.tensor.matmul(h1p[:], lhsT=ccols[:, k:k + 1], rhs=W1p_s[:, k * 256:(k + 1) * 256],
                                 start=(k == 0), stop=False)
            nc.tensor.matmul(h1p[:], lhsT=one11[:], rhs=b1r_s[:], start=False, stop=True)
            h1r = sb.tile([1, 256], f16)
            nc.scalar.activation(h1r[:], h1p[:], AF.Relu)
            h1c = sb.tile([128, 2], f16)
            for k in range(2):
                nc.vector.tensor_copy(h1c[:, k:k + 1], row2col(h1r[0:1, k * 128:(k + 1) * 128])[:])
            W2p_s, b2r_s = load("W2p"), load("b2r")
            h2p = ps.tile([1, 128], f32, space="PSUM", tag="mm")
            for k in range(2):
                nc.tensor.matmul(h2p[:], lhsT=h1c[:, k:k + 1], rhs=W2p_s[:, k * 128:(k + 1) * 128],
                                 start=(k == 0), stop=False)
            nc.tensor.matmul(h2p[:], lhsT=one11[:], rhs=b2r_s[:], start=False, stop=True)
            h2r = sb.tile([1, 128], f16)
            nc.scalar.activation(h2r[:], h2p[:], AF.Relu)
            h2c = sb.tile([128, 1], f16)
            nc.vector.tensor_copy(h2c[:], row2col(h2r[0:1, :])[:])
            W3sl_s, b3sl_s = load("W3sl"), load("b3sl")
            ntr = sb.tile([1, TSLP], f32)
            for j in range(TSLP // 512):
                ntp = ps.tile([1, 512], f32, space="PSUM", tag="mm")
                nc.tensor.matmul(ntp[:], lhsT=h2c[:], rhs=W3sl_s[:, j * 512:(j + 1) * 512], start=True, stop=False)
                nc.tensor.matmul(ntp[:], lhsT=one11[:], rhs=b3sl_s[0:1, j * 512:(j + 1) * 512], start=False, stop=True)
                nc.scalar.activation(ntr[0:1, j * 512:(j + 1) * 512], ntp[:], AF.Sigmoid)
            nc.sync.dma_start(out=nt_o[:], in_=ntr[:])
            enr = sb.tile([1, TSLP], f32)
            nc.vector.tensor_scalar(out=enr[:], in0=viol[:], scalar1=0.0, scalar2=None, op0=ALU.is_equal)
            nc.sync.dma_start(out=en_o[:], in_=enr[:])
            # conf partial
            ntc = sb.tile([128, 20], f32)
            nc.sync.dma_start(out=ntc[:], in_=ntr[:])
            ntc16 = sb.tile([128, 20], f16)
            nc.vector.tensor_copy(ntc16[:], ntc[:])
            enc = sb.tile([128, 20], f32)
            nc.sync.dma_start(out=enc[:], in_=enr[:])
            enc16 = sb.tile([128, 20], f16)
            nc.vector.tensor_copy(enc16[:], enc[:])
            Wc1nt_s, Wc1en_s = load("Wc1nt"), load("Wc1en")
            cp = ps.tile([1, 256], f32, space="PSUM", tag="mm")
            for k in range(20):
                nc.tensor.matmul(cp[:], lhsT=ntc16[:, k:k + 1], rhs=Wc1nt_s[:, k * 256:(k + 1) * 256],
                                 start=(k == 0), stop=False)
            for k in range(20):
                nc.tensor.matmul(cp[:], lhsT=enc16[:, k:k + 1], rhs=Wc1en_s[:, k * 256:(k + 1) * 256],
                                 start=False, stop=(k == 19))
            cpr = sb.tile([1, 256], f32)
            nc.vector.tensor_copy(cpr[:], cp[:])
            nc.sync.dma_start(out=arc_i[:], in_=cpr[:])
            nc.gpsimd.collective_compute("AllReduce", ALU.add, replica_groups=RG,
                                         ins=[arc_i[:]], outs=[arc_o[:]])
            cprr = sb.tile([1, 256], f32)
            nc.sync.dma_start(out=cprr[:], in_=arc_o[:])
            Wc1c_s, bc1r_s = load("Wc1c"), load("bc1r")
            cc = ps.tile([1, 256], f32, space="PSUM", tag="mm")
            for k in range(3):
                nc.tensor.matmul(cc[:], lhsT=ccols[:, k:k + 1], rhs=Wc1c_s[:, k * 256:(k + 1) * 256],
                                 start=(k == 0), stop=False)
            nc.tensor.matmul(cc[:], lhsT=one11[:], rhs=bc1r_s[:], start=False, stop=True)
            c1s = sb.tile([1, 256], f32)
            nc.vector.tensor_tensor(out=c1s[:], in0=cprr[:], in1=cc[:], op=ALU.add)
            c1r = sb.tile([1, 256], f16)
            nc.scalar.activation(c1r[:], c1s[:], AF.Relu)
            c1c = sb.tile([128, 2], f16)
            for k in range(2):
                nc.vector.tensor_copy(c1c[:, k:k + 1], row2col(c1r[0:1, k * 128:(k + 1) * 128])[:])
            Wc2p_s, bc2r_s = load("Wc2p"), load("bc2r")
            c2p = ps.tile([1, 128], f32, space="PSUM", tag="mm")
            for k in range(2):
                nc.tensor.matmul(c2p[:], lhsT=c1c[:, k:k + 1], rhs=Wc2p_s[:, k * 128:(k + 1) * 128],
                                 start=(k == 0), stop=False)
            nc.tensor.matmul(c2p[:], lhsT=one11[:], rhs=bc2r_s[:], start=False, stop=True)
            c2r = sb.tile([1, 128], f16)
            nc.scalar.activation(c2r[:], c2p[:], AF.Relu)
            c2c = sb.tile([128, 1], f16)
            nc.vector.tensor_copy(c2c[:], row2col(c2r[0:1, :])[:])
            Wc3_s, bc3_s = load("Wc3c"), load("bc3")
            c3p = ps.tile([1, 1], f32, space="PSUM", tag="mm")
            nc.tensor.matmul(c3p[:], lhsT=c2c[:], rhs=Wc3_s[:], start=True, stop=False)
            nc.tensor.matmul(c3p[:], lhsT=one11[:], rhs=bc3_s[:], start=False, stop=True)
            cfr = sb.tile([1, 1], f32)
            nc.scalar.activation(cfr[:], c3p[:], AF.Sigmoid)
            nc.sync.dma_start(out=cf_o[:], in_=cfr[:])

    res = run_bass_kernel_spmd(nc, per_core_in, core_ids=list(range(NC)))
    kernel._last_res = res
    nt = np.concatenate([res.results[c]["nt"][0, :TSL] for c in range(NC)]).astype(np.float32)
    en = np.concatenate([res.results[c]["en"][0, :TSL] for c in range(NC)]).astype(np.float32)
    cf = res.results[0]["cf"].reshape(1).astype(np.float32)
    return nt, cf, en


# revision 7
# speedup vs baseline: 1.6633x; 1.6633x over previous
"""Trainium2 Bass kernel for the ConformanceGNN (3-layer bipartite GNN message passing).

Sharding: dest-sharded edge parallelism over 8 cores. Each core owns a slice of the
destination nodes per direction, gathers source-node rows (feat|score|bad packed in
512B fp16 table rows) with indirect DMA, segment-sums via one-hot matmuls into PSUM,
applies the fused update (message MLP folded into derived weights host-side), and
AllGathers the updated node table for the next pass. Global softmax normalizer Z and
max-shift m are computed densely on every core from the replicated score column.
"""
import numpy as np

P, T, H, L = 50000, 20000, 128, 3
NC = 8
PSL, PSLP = P // NC, 6400          # place slice, padded (50 tiles)
TSL, TSLP = T // NC, 2560          # trans slice, padded (20 tiles)
PTILES, TTILES = PSLP // 128, TSLP // 128
PROWS, TROWS = NC * PSLP, NC * TSLP  # replicated table rows


def _patch_tile_drain():
    """walrus here rejects >1 sem wait on the closing Drain; split waits onto NOPs."""
    import concourse.tile as tile
    import concourse.mybir as mybir
    from concourse.vector_clock import ScopedClock

    def _drain_and_barrier_split(self, tick_clock, wait_clock):
        probe = self.nc.sync.nop(nofuse=True)
        wait_clock.add_sem_waits(probe.ins, ScopedClock({None: tick_clock.global_clock}))
        si = probe.ins.sync_info
        waits = list(si.on_wait) if si is not None else []
        if len(waits) > 1:
            probe.ins.sync_info = mybir.SyncInfo(on_wait=waits[:1], on_update=[])
            for w in waits[1:]:
                n = self.nc.sync.nop(nofuse=True)
                n.ins.sync_info = mybir.SyncInfo(on_wait=[w], on_update=[])
        self.nc.sync.drain()
        self.nc.all_engine_barrier()
        assert self.sems is not None
        popped = self.nc._tile_sem_poison_stack.pop()
        assert popped is self._sem_poison
        self.nc.clear_and_free_semaphores(list(self.sems.allocated().values()))
        self.nc.all_engine_barrier()

    tile.TileContext._drain_and_barrier = _drain_and_barrier_split

    if getattr(tile.TileContext, "_wait_split_patched", False):
        return
    orig_commit = tile.TileContext._commit_instruction

    def _commit_split(self, inst, lazy_reg_writes=True):
        si = getattr(inst, "sync_info", None)
        if (si is not None and si.on_wait and len(si.on_wait) > 1
                and inst.engine != mybir.EngineType.Unassigned):
            waits = list(si.on_wait)
            inst.sync_info = mybir.SyncInfo(on_wait=[waits[-1]], on_update=list(si.on_update))
            cb = self.nc._state.pop_inst_callback()
            try:
                eng = self.nc.engines[inst.engine]
                for w in waits[:-1]:
                    n = eng.nop(nofuse=True)
                    n.ins.sync_info = mybir.SyncInfo(on_wait=[w], on_update=[])
            finally:
                self.nc._state.push_inst_callback(cb)
        orig_commit(self, inst, lazy_reg_writes)

    tile.TileContext._commit_instruction = _commit_split
    tile.TileContext._wait_split_patched = True


def _prep_edges(src, dst, src_slp, dst_sl, dst_slp, src_rowmap):
    """Per-core chunked edge lists. Returns idx [NC][128, NCHtot] i32 (table rows),
    dloc [NC][128, NCHtot] f32 (dest pos in tile, -1 pad), nch [ntiles]."""
    ntiles = dst_slp // 128
    per_core = []
    for c in range(NC):
        m = (dst >= c * dst_sl) & (dst < (c + 1) * dst_sl)
        es, ed = src[m], dst[m] - c * dst_sl
        tiles = [[] for _ in range(ntiles)]
        order = np.argsort(ed // 128, kind="stable")
        for e in order:
            tiles[ed[e] // 128].append((es[e], ed[e] % 128))
        per_core.append(tiles)
    nch = np.array([max(1, max((len(per_core[c][t]) + 127) // 128 for c in range(NC)))
                    for t in range(ntiles)])
    tot = int(nch.sum())
    idx = np.zeros((NC, 128, tot), np.int32)
    dloc = np.full((NC, 128, tot), -1.0, np.float32)
    for c in range(NC):
        k0 = 0
        for t in range(ntiles):
            lst = per_core[c][t]
            for j, (s, dp) in enumerate(lst):
                idx[c, j % 128, k0 + j // 128] = src_rowmap(s)
                dloc[c, j % 128, k0 + j // 128] = dp
            k0 += nch[t]
    return idx, dloc, nch


def kernel(**inputs):
    import concourse.bass as bass
    import concourse.mybir as mybir
    import concourse.tile as tile
    from concourse.bass_utils import run_bass_kernel_spmd
    _patch_tile_drain()
    f16, f32, i32 = mybir.dt.float16, mybir.dt.float32, mybir.dt.int32
    AF = mybir.ActivationFunctionType
    ALU = mybir.AluOpType

    d = {k: np.asarray(v) for k, v in inputs.items()}

    # ---- host: derived weights -------------------------------------------------
    def f16a(x):
        return np.ascontiguousarray(x, np.float16)

    def f32a(x):
        return np.ascontiguousarray(x, np.float32)

    Wsq, bvecs, bcols, ucols = [], [], [], []
    for l in range(L):
        Wp2t, bp2t = d["Wp2t"][l], d["bp2t"][l]
        Wt2p, bt2p = d["Wt2p"][l], d["bt2p"][l]
        Wpu, bpu = d["Wpu"][l], d["bpu"][l]
        Wtu, btu = d["Wtu"][l], d["btu"][l]
        I = np.eye(H, dtype=np.float32)
        Wsq += [I + Wtu[:H], Wp2t @ Wtu[H:], I + Wpu[:H], Wt2p @ Wpu[H:]]
        bvecs += [bp2t @ Wtu[H:], bt2p @ Wpu[H:]]
        bcols += [btu, bpu]
        ucols.append(d["Wt2p"][l] @ d["Wpa"][l])          # u_post[l]: trans scores
    for l in (1, 2):
        ucols.append(d["Wp2t"][l] @ d["Wta"][l])          # u_pre[l]: place scores
    u_pre0 = d["Wp2t"][0] @ d["Wta"][0]
    alpha_p = float(d["W_pe"][0] @ u_pre0)
    beta_p = float(d["b_pe"] @ u_pre0)
    vt = d["W_te"] @ ucols[0]                              # [8] trans embed score
    beta_t = float(d["b_te"] @ ucols[0])

    # ---- host: edges / tables --------------------------------------------------
    pre, post = np.asarray(d["pre_edge_index"]), np.asarray(d["post_edge_index"])
    prow = lambda p: (p // PSL) * PSLP + p % PSL
    trow = lambda t: (t // TSL) * TSLP + t % TSL
    idxA, dlocA, nchA = _prep_edges(pre[0].astype(np.int64), pre[1].astype(np.int64),
                                    PSLP, TSL, TSLP, prow)
    idxB, dlocB, nchB = _prep_edges(post[0].astype(np.int64), post[1].astype(np.int64),
                                    TSLP, PSL, PSLP, trow)
    NA, NB = idxA.shape[2], idxB.shape[2]

    cntP = np.zeros(PROWS, np.float32)
    np.add.at(cntP, prow(pre[0].astype(np.int64)), 1.0)
    cntT = np.zeros(TROWS, np.float32)
    np.add.at(cntT, trow(post[0].astype(np.int64)), 1.0)
    CP, CT = PROWS // 128, TROWS // 128

    pf = d["place_features"].reshape(-1).astype(np.float32)
    tf = d["transition_features"].astype(np.float32)
    bad = (pf <= 0).astype(np.float16)

    per_core_in = []
    for c in range(NC):
        pfs = np.zeros(PSLP, np.float32)
        pfs[:PSL] = pf[c * PSL:(c + 1) * PSL]
        bads = np.zeros(PSLP, np.float16)
        bads[:PSL] = bad[c * PSL:(c + 1) * PSL]
        tfs = np.zeros((TSLP, 8), np.float32)
        tfs[:TSL] = tf[c * TSL:(c + 1) * TSL]
        W3sl = np.zeros((H, TSLP), np.float16)
        W3sl[:, :TSL] = d["W3"][:, c * TSL:(c + 1) * TSL].astype(np.float16)
        b3sl = np.zeros((1, TSLP), np.float16)
        b3sl[0, :TSL] = d["b3"][c * TSL:(c + 1) * TSL].astype(np.float16)
        Wc1nt = np.zeros((128, 20, 256), np.float16)
        Wc1en = np.zeros((128, 20, 256), np.float16)
        for p in range(128):
            for k in range(20):
                n = p * 20 + k
                if n < TSL:
                    Wc1nt[p, k] = d["Wc1"][3 * H + c * TSL + n].astype(np.float16)
                    Wc1en[p, k] = d["Wc1"][3 * H + T + c * TSL + n].astype(np.float16)
        m = {
            "idxA": idxA[c], "dlocA": f32a(dlocA[c]),
            "idxB": idxB[c], "dlocB": f32a(dlocB[c]),
            "cntP": f32a(cntP.reshape(128, CP)), "cntT": f32a(cntT.reshape(128, CT)),
            "pfrow": f16a(pfs.reshape(1, -1)), "pfcol": f32a(pfs.reshape(128, PTILES, order="F").reshape(128, PTILES)),
            "badc": f16a(bads.reshape(128, PTILES, order="F")),
            "tfT": f16a(tfs.T), "prefc": f16a(d["prefix_encoding"].reshape(-1, 1)),
            "iota": f16a(np.tile(np.arange(128, dtype=np.float16)[None, :], (128, 1))),
            "ident": f16a(np.eye(128)), "onesr": f16a(np.ones((1, 128))),
            "onesrf": f32a(np.ones((1, 128))), "one11": f16a(np.ones((1, 1))),
            "zcol": f16a(np.zeros((128, 1))),
            "pmaskp": f32a((np.arange(PSLP).reshape(PTILES, 128).T < PSL).astype(np.float32)),
            "pmaskt": f32a((np.arange(TSLP).reshape(TTILES, 128).T < TSL).astype(np.float32)),
            "Wsq": f16a(np.concatenate(Wsq, 1)), "bvecs": f16a(np.concatenate([b.reshape(1, -1) for b in bvecs], 1)),
            "bcols": f32a(np.stack(bcols, 1)), "ucols": f16a(np.stack(ucols, 1)),
            "Wpe": f16a(d["W_pe"]), "bpe": f16a(d["b_pe"].reshape(1, -1)),
            "Wte": f16a(d["W_te"]), "bte": f16a(d["b_te"].reshape(1, -1)),
            "vtc": f16a(vt.reshape(-1, 1)),
            "Wpp": f16a(d["Wpp"]), "bpp": f16a(d["bpp"].reshape(1, -1)),
            "Wtp": f16a(d["Wtp"]), "btp": f16a(d["btp"].reshape(1, -1)),
            "Wpx": f16a(d["W_px"]), "bpx": f16a(d["b_px"].reshape(1, -1)),
            "W1p": f16a(d["W1"].reshape(3, 128, 256).transpose(1, 0, 2).reshape(128, 768)),
            "b1r": f16a(d["b1"].reshape(1, -1)),
            "W2p": f16a(d["W2"].reshape(2, 128, 128).transpose(1, 0, 2).reshape(128, 256)),
            "b2r": f16a(d["b2"].reshape(1, -1)),
            "W3sl": W3sl, "b3sl": b3sl,
            "Wc1c": f16a(d["Wc1"][:384].reshape(3, 128, 256).transpose(1, 0, 2).reshape(128, 768)),
            "bc1r": f16a(d["bc1"].reshape(1, -1)),
            "Wc1nt": Wc1nt.reshape(128, 5120), "Wc1en": Wc1en.reshape(128, 5120),
            "Wc2p": f16a(d["Wc2"].reshape(2, 128, 128).transpose(1, 0, 2).reshape(128, 256)),
            "bc2r": f16a(d["bc2"].reshape(1, -1)),
            "Wc3c": f16a(d["Wc3"]), "bc3": f16a(d["bc3"].reshape(1, 1)),
        }
        per_core_in.append(m)

    # pfcol/badc pack: node n of slice at (p=n%128, t=n//128)
    for m in per_core_in:
        pfs = m["pfrow"].reshape(-1).astype(np.float32)
        m["pfcol"] = f32a(pfs.reshape(PTILES, 128).T)
        bads = m["badc"]  # placeholder fix below
    for c in range(NC):
        bads = np.zeros(PSLP, np.float16)
        bads[:PSL] = bad[c * PSL:(c + 1) * PSL]
        per_core_in[c]["badc"] = f16a(bads.reshape(PTILES, 128).T)

    # ---- bass program ----------------------------------------------------------
    nc = bass.Bass(num_devices=NC)
    ins = {}
    for k, v in per_core_in[0].items():
        dt = {np.dtype(np.float16): f16, np.dtype(np.float32): f32,
              np.dtype(np.int32): i32}[v.dtype]
        ins[k] = nc.dram_tensor(k, list(v.shape), dt, kind="ExternalInput")
    nt_o = nc.dram_tensor("nt", [1, TSLP], f32, kind="ExternalOutput")
    en_o = nc.dram_tensor("en", [1, TSLP], f32, kind="ExternalOutput")
    cf_o = nc.dram_tensor("cf", [1, 1], f32, kind="ExternalOutput")

    slabP = [nc.dram_tensor(f"slabP{i}", [PSLP, 256], f16) for i in range(4)]
    slabT = [nc.dram_tensor(f"slabT{i}", [TSLP, 256], f16) for i in range(4)]
    tabP = [nc.dram_tensor(f"tabP{i}", [PROWS, 256], f16, addr_space="Shared") for i in range(3)]
    tabT = [nc.dram_tensor(f"tabT{i}", [TROWS, 256], f16, addr_space="Shared") for i in range(3)]
    arm_i = nc.dram_tensor("arm_i", [128, 2], f32)
    arm_o = nc.dram_tensor("arm_o", [128, 2], f32, addr_space="Shared")
    arc_i = nc.dram_tensor("arc_i", [1, 256], f32)
    arc_o = nc.dram_tensor("arc_o", [1, 256], f32, addr_space="Shared")
    RG = [list(range(NC))]

    with tile.TileContext(nc) as tc:
        with (
            tc.tile_pool(name="pers", bufs=1) as pers,
            tc.tile_pool(name="sb", bufs=4) as sb,
            tc.tile_pool(name="sw", bufs=4) as sw,
            tc.tile_pool(name="ps", bufs=2, space="PSUM") as ps,
            tc.tile_pool(name="pacc", bufs=2, space="PSUM") as pacc,
        ):
            # persistent/consts
            def load(name, dt=None):
                t = pers.tile(list(per_core_in[0][name].shape), dt or ins[name].dtype, name=name)
                nc.sync.dma_start(out=t[:], in_=ins[name][:])
                return t
            iota, ident = load("iota"), load("ident")
            onesr, onesrf, one11, zcol = load("onesr"), load("onesrf"), load("one11"), load("zcol")
            idxA_s, dlocA_s = load("idxA"), load("dlocA")
            idxB_s, dlocB_s = load("idxB"), load("dlocB")
            pfrow, pfcol, badc, tfT = load("pfrow"), load("pfcol"), load("badc"), load("tfT")
            pmaskp, pmaskt = load("pmaskp"), load("pmaskt")
            Wsq_s, bvecs_s, bcols_s, ucols_s = load("Wsq"), load("bvecs"), load("bcols"), load("ucols")
            cntP_s, cntT_s = load("cntP"), load("cntT")
            viol = pers.tile([1, TSLP], f32)
            acc_p = pers.tile([128, 1], f32)
            acc_t = pers.tile([128, 1], f32)
            nc.vector.memset(acc_p[:], 0.0)
            nc.vector.memset(acc_t[:], 0.0)

            def row2col(row_ap, n=128):
                """[1,n] -> psum [n,1] via matmul with one11."""
                pt = ps.tile([128, 1], f32, space="PSUM", tag="mm")
                nc.tensor.matmul(pt[:n, :], lhsT=row_ap, rhs=one11[:], start=True, stop=True)
                return pt

            def bcast_col(v11_f32):
                """[1,1] f32 -> [128,1] f32 col (psum)."""
                pt = ps.tile([128, 1], f32, space="PSUM", tag="mm")
                nc.tensor.matmul(pt[:], lhsT=onesrf[:], rhs=v11_f32, start=True, stop=True)
                return pt

            # ---------- embeds ----------
            Wpe_s, bpe_s = load("Wpe"), load("bpe")
            for t in range(PTILES):
                pe = ps.tile([128, 128], f32, space="PSUM", tag="mm")
                nc.tensor.matmul(pe[:], lhsT=pfrow[0:1, t * 128:(t + 1) * 128], rhs=Wpe_s[:], start=True, stop=False)
                nc.tensor.matmul(pe[:], lhsT=onesr[:], rhs=bpe_s[:], start=False, stop=True)
                wt = sw.tile([128, 256], f16)
                nc.vector.tensor_copy(wt[:, 0:128], pe[:])
                nc.scalar.activation(wt[:, 128:129], pfcol[:, t:t + 1], AF.Copy, bias=beta_p, scale=alpha_p)
                nc.vector.tensor_copy(wt[:, 129:130], badc[:, t:t + 1])
                if (t + 1) * 128 > PSL:
                    nc.vector.tensor_scalar_mul(wt[:, 0:130], wt[:, 0:130], pmaskp[:, t:t + 1])
                nc.sync.dma_start(out=slabP[0][t * 128:(t + 1) * 128, :], in_=wt[:])
            Wte_s, bte_s, vtc_s = load("Wte"), load("bte"), load("vtc")
            for t in range(TTILES):
                pe = ps.tile([128, 128], f32, space="PSUM", tag="mm")
                nc.tensor.matmul(pe[:], lhsT=tfT[:, t * 128:(t + 1) * 128], rhs=Wte_s[:], start=True, stop=False)
                nc.tensor.matmul(pe[:], lhsT=onesr[:], rhs=bte_s[:], start=False, stop=True)
                sp = ps.tile([128, 1], f32, space="PSUM", tag="mm")
                nc.tensor.matmul(sp[:], lhsT=tfT[:, t * 128:(t + 1) * 128], rhs=vtc_s[:], start=True, stop=True)
                wt = sw.tile([128, 256], f16)
                nc.vector.tensor_copy(wt[:, 0:128], pe[:])
                nc.scalar.activation(wt[:, 128:129], sp[:], AF.Copy, bias=beta_t)
                nc.vector.tensor_copy(wt[:, 129:130], zcol[:])
                if (t + 1) * 128 > TSL:
                    nc.vector.tensor_scalar_mul(wt[:, 0:130], wt[:, 0:130], pmaskt[:, t:t + 1])
                nc.sync.dma_start(out=slabT[0][t * 128:(t + 1) * 128, :], in_=wt[:])
            nc.gpsimd.collective_compute("AllGather", ALU.bypass, replica_groups=RG,
                                         ins=[slabP[0][:]], outs=[tabP[0][:]])

            # ---------- passes ----------
            def mz(tab, cnt_s, C):
                """returns (mneg_col f32 sbuf [128,1], zinv_col f32 sbuf [128,1], zinv11 sbuf [1,1])"""
                s_sb = sb.tile([128, C], f16)
                nc.sync.dma_start(out=s_sb[:], in_=tab[:, 128:129])
                m11 = sb.tile([1, 1], f32)
                nc.gpsimd.tensor_reduce(m11[:], s_sb[:], mybir.AxisListType.XYZWC, ALU.max)
                mneg11 = sb.tile([1, 1], f32)
                nc.vector.tensor_scalar_mul(mneg11[:], m11[:], -1.0)
                mnc = sb.tile([128, 1], f32)
                nc.vector.tensor_copy(mnc[:], bcast_col(mneg11[:])[:])
                zt = sb.tile([128, C], f32)
                nc.scalar.activation(zt[:], s_sb[:], AF.Exp, bias=mnc[:])
                nc.vector.tensor_tensor(out=zt[:], in0=zt[:], in1=cnt_s[:], op=ALU.mult)
                z11 = sb.tile([1, 1], f32)
                nc.gpsimd.tensor_reduce(z11[:], zt[:], mybir.AxisListType.XYZWC, ALU.add)
                zi11 = sb.tile([1, 1], f32)
                nc.vector.reciprocal(zi11[:], z11[:])
                zic = sb.tile([128, 1], f32)
                nc.vector.tensor_copy(zic[:], bcast_col(zi11[:])[:])
                return mnc, zic, zi11

            def gnn_pass(l, is_pre):
                tab = tabP[l] if is_pre else tabT[l]
                cnt_s, C = (cntP_s, CP) if is_pre else (cntT_s, CT)
                idx_s, dloc_s, nch = (idxA_s, dlocA_s, nchA) if is_pre else (idxB_s, dlocB_s, nchB)
                ntl, SL = (TTILES, TSL) if is_pre else (PTILES, PSL)
                rslab, wslab = (slabT[l], slabT[l + 1]) if is_pre else (slabP[l], slabP[l + 1])
                wi = (4 * l + (0 if is_pre else 2))
                W1s = sb.tile([128, 128], f16)
                nc.sync.dma_start(out=W1s[:], in_=ins["Wsq"][:, wi * 128:(wi + 1) * 128])
                W2s = sb.tile([128, 128], f16)
                nc.sync.dma_start(out=W2s[:], in_=ins["Wsq"][:, (wi + 1) * 128:(wi + 2) * 128])
                bi = 2 * l + (0 if is_pre else 1)
                mnc, zic, zi11 = mz(tab, cnt_s, C)
                ui = l if is_pre else (2 + l)   # ucols index; post l=2 has none
                k0 = 0
                for t in range(ntl):
                    stf = pacc.tile([128, 128], f32, space="PSUM", tag="stf")
                    ext = pacc.tile([1, 128], f32, space="PSUM", tag="ext")
                    need_viol = is_pre and l == 0
                    if need_viol:
                        vlp = pacc.tile([1, 128], f32, space="PSUM", tag="viol", bufs=1)
                    for k in range(k0, k0 + int(nch[t])):
                        g = sb.tile([128, 256], f16)
                        nc.gpsimd.indirect_dma_start(
                            out=g[:], out_offset=None, in_=tab[:],
                            in_offset=bass.IndirectOffsetOnAxis(ap=idx_s[:, k:k + 1], axis=0))
                        w = sb.tile([128, 1], f32)
                        nc.scalar.activation(w[:], g[:, 128:129], AF.Exp, bias=mnc[:])
                        nc.vector.tensor_scalar_mul(g[:, 0:128], g[:, 0:128], w[:])
                        nc.vector.tensor_copy(g[:, 128:129], w[:])
                        meq = sb.tile([128, 128], f16)
                        nc.vector.tensor_scalar(out=meq[:], in0=iota[:], scalar1=dloc_s[:, k:k + 1],
                                                scalar2=None, op0=ALU.is_equal)
                        st, sp_ = (k == k0), (k == k0 + int(nch[t]) - 1)
                        nc.tensor.matmul(stf[:], lhsT=g[:, 0:128], rhs=meq[:], start=st, stop=sp_)
                        nc.tensor.matmul(ext[:], lhsT=g[:, 128:129], rhs=meq[:], start=st, stop=sp_)
                        if need_viol:
                            nc.tensor.matmul(vlp[:], lhsT=g[:, 129:130], rhs=meq[:], start=st, stop=sp_)
                    k0 += int(nch[t])
                    if need_viol:
                        nc.vector.tensor_copy(viol[0:1, t * 128:(t + 1) * 128], vlp[:])
                    # update
                    ld = sb.tile([128, 128], f16)
                    nc.sync.dma_start(out=ld[:], in_=rslab[t * 128:(t + 1) * 128, 0:128])
                    tp = ps.tile([128, 128], f32, space="PSUM", tag="mm")
                    nc.tensor.matmul(tp[:], lhsT=ld[:], rhs=ident[:], start=True, stop=True)
                    hT = sb.tile([128, 128], f16)
                    nc.vector.tensor_copy(hT[:], tp[:])
                    upd = ps.tile([128, 128], f32, space="PSUM", tag="mm")
                    nc.tensor.matmul(upd[:], lhsT=W1s[:], rhs=hT[:], start=True, stop=False)
                    sts = sb.tile([128, 128], f16)
                    nc.vector.tensor_scalar_mul(sts[:], stf[:], zic[:])
                    nc.tensor.matmul(upd[:], lhsT=W2s[:], rhs=sts[:], start=False, stop=False)
                    satt = sb.tile([1, 128], f16)
                    nc.vector.tensor_scalar_mul(satt[:], ext[:], zi11[:])
                    nc.tensor.matmul(upd[:], lhsT=bvecs_s[0:1, bi * 128:(bi + 1) * 128], rhs=satt[:],
                                     start=False, stop=True)
                    hN = sb.tile([128, 128], f16)
                    nc.scalar.activation(hN[:], upd[:], AF.Relu, bias=bcols_s[:, bi:bi + 1])
                    if (t + 1) * 128 > SL:
                        nc.vector.memset(hN[:, max(0, SL - t * 128):128], 0.0)
                    if l == 2:
                        rs = sb.tile([128, 1], f32)
                        nc.vector.tensor_reduce(rs[:], hN[:], mybir.AxisListType.X, ALU.add)
                        acc = acc_t if is_pre else acc_p
                        nc.vector.tensor_tensor(out=acc[:], in0=acc[:], in1=rs[:], op=ALU.add)
                    wp = ps.tile([128, 128], f32, space="PSUM", tag="mm")
                    nc.tensor.matmul(wp[:], lhsT=hN[:], rhs=ident[:], start=True, stop=True)
                    wt = sw.tile([128, 256], f16)
                    nc.vector.tensor_copy(wt[:, 0:128], wp[:])
                    if not (not is_pre and l == 2):
                        sp = ps.tile([128, 1], f32, space="PSUM", tag="mm")
                        nc.tensor.matmul(sp[:], lhsT=hN[:], rhs=ucols_s[:, ui:ui + 1], start=True, stop=True)
                        nc.vector.tensor_copy(wt[:, 128:129], sp[:])
                    else:
                        nc.vector.tensor_copy(wt[:, 128:129], zcol[:])
                    if is_pre:
                        nc.vector.tensor_copy(wt[:, 129:130], zcol[:])
                    else:
                        nc.vector.tensor_copy(wt[:, 129:130], badc[:, t:t + 1])
                    nc.sync.dma_start(out=wslab[t * 128:(t + 1) * 128, :], in_=wt[:])
                # allgather updated table
                if is_pre:
                    nc.gpsimd.collective_compute("AllGather", ALU.bypass, replica_groups=RG,
                                                 ins=[wslab[:]], outs=[tabT[l][:]])
                elif l < 2:
                    nc.gpsimd.collective_compute("AllGather", ALU.bypass, replica_groups=RG,
                                                 ins=[wslab[:]], outs=[tabP[l + 1][:]])

            for l in range(L):
                gnn_pass(l, True)
                gnn_pass(l, False)

            # ---------- head ----------
            mv = sb.tile([128, 2], f32)
            nc.vector.tensor_copy(mv[:, 0:1], acc_p[:])
            nc.vector.tensor_copy(mv[:, 1:2], acc_t[:])
            nc.sync.dma_start(out=arm_i[:], in_=mv[:])
            nc.gpsimd.collective_compute("AllReduce", ALU.add, replica_groups=RG,
                                         ins=[arm_i[:]], outs=[arm_o[:]])
            mvr = sb.tile([128, 2], f32)
            nc.sync.dma_start(out=mvr[:], in_=arm_o[:])
            mcols = sb.tile([128, 2], f16)
            nc.vector.tensor_scalar_mul(mcols[:, 0:1], mvr[:, 0:1], 1.0 / P)
            nc.vector.tensor_scalar_mul(mcols[:, 1:2], mvr[:, 1:2], 1.0 / T)
            Wpp_s, bpp_s, Wtp_s, btp_s = load("Wpp"), load("bpp"), load("Wtp"), load("btp")
            Wpx_s, bpx_s, prefc = load("Wpx"), load("bpx"), load("prefc")
            comb = sb.tile([1, 384], f16)
            pg = ps.tile([1, 128], f32, space="PSUM", tag="mm")
            nc.tensor.matmul(pg[:], lhsT=mcols[:, 0:1], rhs=Wpp_s[:], start=True, stop=False)
            nc.tensor.matmul(pg[:], lhsT=one11[:], rhs=bpp_s[:], start=False, stop=True)
            nc.vector.tensor_copy(comb[0:1, 0:128], pg[:])
            tg = ps.tile([1, 128], f32, space="PSUM", tag="mm")
            nc.tensor.matmul(tg[:], lhsT=mcols[:, 1:2], rhs=Wtp_s[:], start=True, stop=False)
            nc.tensor.matmul(tg[:], lhsT=one11[:], rhs=btp_s[:], start=False, stop=True)
            nc.vector.tensor_copy(comb[0:1, 128:256], tg[:])
            ph = ps.tile([1, 128], f32, space="PSUM", tag="mm")
            nc.tensor.matmul(ph[:], lhsT=prefc[:], rhs=Wpx_s[:], start=True, stop=False)
            nc.tensor.matmul(ph[:], lhsT=one11[:], rhs=bpx_s[:], start=False, stop=True)
            nc.vector.tensor_copy(comb[0:1, 256:384], ph[:])
            ccols = sb.tile([128, 3], f16)
            for k in range(3):
                nc.vector.tensor_copy(ccols[:, k:k + 1], row2col(comb[0:1, k * 128:(k + 1) * 128])[:])
            W1p_s, b1r_s = load("W1p"), load("b1r")
            h1p = ps.tile([1, 256], f32, space="PSUM", tag="mm")
            for k in range(3):
                nc.tensor.matmul(h1p[:], lhsT=ccols[:, k:k + 1], rhs=W1p_s[:, k * 256:(k + 1) * 256],
                                 start=(k == 0), stop=False)
            nc.tensor.matmul(h1p[:], lhsT=one11[:], rhs=b1r_s[:], start=False, stop=True)
            h1r = sb.tile([1, 256], f16)
            nc.scalar.activation(h1r[:], h1p[:], AF.Relu)
            h1c = sb.tile([128, 2], f16)
            for k in range(2):
                nc.vector.tensor_copy(h1c[:, k:k + 1], row2col(h1r[0:1, k * 128:(k + 1) * 128])[:])
            W2p_s, b2r_s = load("W2p"), load("b2r")
            h2p = ps.tile([1, 128], f32, space="PSUM", tag="mm")
            for k in range(2):
                nc.tensor.matmul(h2p[:], lhsT=h1c[:, k:k + 1], rhs=W2p_s[:, k * 128:(k + 1) * 128],
                                 start=(k == 0), stop=False)
            nc.tensor.matmul(h2p[:], lhsT=one11[:], rhs=b2r_s[:], start=False, stop=True)
            h2r = sb.tile([1, 128], f16)
            nc.scalar.activation(h2r[:], h2p[:], AF.Relu)
            h2c = sb.tile([128, 1], f16)
            nc.vector.tensor_copy(h2c[:], row2col(h2r[0:1, :])[:])
            W3sl_s, b3sl_s = load("W3sl"), load("b3sl")
            ntr = sb.tile([1, TSLP], f32)
            for j in range(TSLP // 512):
                ntp = ps.tile([1, 512], f32, space="PSUM", tag="mm")
                nc.tensor.matmul(ntp[:], lhsT=h2c[:], rhs=W3sl_s[:, j * 512:(j + 1) * 512], start=True, stop=False)
                nc.tensor.matmul(ntp[:], lhsT=one11[:], rhs=b3sl_s[0:1, j * 512:(j + 1) * 512], start=False, stop=True)
                nc.scalar.activation(ntr[0:1, j * 512:(j + 1) * 512], ntp[:], AF.Sigmoid)
            nc.sync.dma_start(out=nt_o[:], in_=ntr[:])
            enr = sb.tile([1, TSLP], f32)
            nc.vector.tensor_scalar(out=enr[:], in0=viol[:], scalar1=0.0, scalar2=None, op0=ALU.is_equal)
            nc.sync.dma_start(out=en_o[:], in_=enr[:])
            # conf partial
            ntc = sb.tile([128, 20], f32)
            nc.sync.dma_start(out=ntc[:], in_=ntr[:])
            ntc16 = sb.tile([128, 20], f16)
            nc.vector.tensor_copy(ntc16[:], ntc[:])
            enc = sb.tile([128, 20], f32)
            nc.sync.dma_start(out=enc[:], in_=enr[:])
            enc16 = sb.tile([128, 20], f16)
            nc.vector.tensor_copy(enc16[:], enc[:])
            Wc1nt_s, Wc1en_s = load("Wc1nt"), load("Wc1en")
            cp = ps.tile([1, 256], f32, space="PSUM", tag="mm")
            for k in range(20):
                nc.tensor.matmul(cp[:], lhsT=ntc16[:, k:k + 1], rhs=Wc1nt_s[:, k * 256:(k + 1) * 256],
                                 start=(k == 0), stop=False)
            for k in range(20):
                nc.tensor.matmul(cp[:], lhsT=enc16[:, k:k + 1], rhs=Wc1en_s[:, k * 256:(k + 1) * 256],
                                 start=False, stop=(k == 19))
            cpr = sb.tile([1, 256], f32)
            nc.vector.tensor_copy(cpr[:], cp[:])
            nc.sync.dma_start(out=arc_i[:], in_=cpr[:])
            nc.gpsimd.collective_compute("AllReduce", ALU.add, replica_groups=RG,
                                         ins=[arc_i[:]], outs=[arc_o[:]])
            cprr = sb.tile([1, 256], f32)
            nc.sync.dma_start(out=cprr[:], in_=arc_o[:])
            Wc1c_s, bc1r_s = load("Wc1c"), load("bc1r")
            cc = ps.tile([1, 256], f32, space="PSUM", tag="mm")
            for k in range(3):
                nc.tensor.matmul(cc[:], lhsT=ccols[:, k:k + 1], rhs=Wc1c_s[:, k * 256:(k + 1) * 256],
                                 start=(k == 0), stop=False)
            nc.tensor.matmul(cc[:], lhsT=one11[:], rhs=bc1r_s[:], start=False, stop=True)
            c1s = sb.tile([1, 256], f32)
            nc.vector.tensor_tensor(out=c1s[:], in0=cprr[:], in1=cc[:], op=ALU.add)
            c1r = sb.tile([1, 256], f16)
            nc.scalar.activation(c1r[:], c1s[:], AF.Relu)
            c1c = sb.tile([128, 2], f16)
            for k in range(2):
                nc.vector.tensor_copy(c1c[:, k:k + 1], row2col(c1r[0:1, k * 128:(k + 1) * 128])[:])
            Wc2p_s, bc2r_s = load("Wc2p"), load("bc2r")
            c2p = ps.tile([1, 128], f32, space="PSUM", tag="mm")
            for k in range(2):
                nc.tensor.matmul(c2p[:], lhsT=c1c[:, k:k + 1], rhs=Wc2p_s[:, k * 128:(k + 1) * 128],
                                 start=(k == 0), stop=False)
            nc.tensor.matmul(c2p[:], lhsT=one11[:], rhs=bc2r_s[:], start=False, stop=True)
            c2r = sb.tile([1, 128], f16)
            nc.scalar.activation(c2r[:], c2p[:], AF.Relu)
            c2c = sb.tile([128, 1], f16)
            nc.vector.tensor_copy(c2c[:], row2col(c2r[0:1, :])[:])
            Wc3_s, bc3_s = load("Wc3c"), load("bc3")
            c3p = ps.tile([1, 1], f32, space="PSUM", tag="mm")
            nc.tensor.matmul(c3p[:], lhsT=c2c[:], rhs=Wc3_s[:], start=True, stop=False)
            nc.tensor.matmul(c3p[:], lhsT=one11[:], rhs=bc3_s[:], start=False, stop=True)
            cfr = sb.tile([1, 1], f32)
            nc.scalar.activation(cfr[:], c3p[:], AF.Sigmoid)
            nc.sync.dma_start(out=cf_o[:], in_=cfr[:])

    res = run_bass_kernel_spmd(nc, per_core_in, core_ids=list(range(NC)))
    kernel._last_res = res
    nt = np.concatenate([res.results[c]["nt"][0, :TSL] for c in range(NC)]).astype(np.float32)
    en = np.concatenate([res.results[c]["en"][0, :TSL] for c in range(NC)]).astype(np.float32)
    cf = res.results[0]["cf"].reshape(1).astype(np.float32)
    return nt, cf, en
